# revision 21
# baseline (speedup 1.0000x reference)
"""Trainium2 Bass kernel for BatchedExpertMoEDispatch.

Strategy (expert-parallel, sparse dispatch, bf16 compute):
  - Host computes the routing table from (expert_ids, expert_weights):
    for each expert e the unique token list idx_e and combined coefficient
    coeff_e (duplicate (token, expert) slots merge by summing weights).
  - Core e receives its expert's tokens and weights in bf16, PRE-LAID-OUT
    host-side so every device DMA is fully contiguous (128-partition rows
    with multi-KB runs — no 512B strided descriptors).
  - Each core runs the full FFN for its expert on its tokens:
        gT = Wg.T @ xT ; uT = Wu.T @ xT          (PSUM, fp32 accum)
        hT = silu(gT) * uT                        (ACT Silu + DVE mult, bf16)
        yT = Wd.T @ hT                            (PSUM, fp32 accum)
        outT = yT * coeff (broadcast over partitions), stored bf16
    Matmuls run in bf16 (full PE rate, FWL-accelerated weight loads); the
    token dim is split 512+472 to fit PSUM banks, and both chunks ride the
    same LDWEIGHTS to keep the PE streaming back-to-back.
  - A short burst of dummy matmuls on zeroed SBUF warms the PE clock gate
    (HAM) while the first real DMAs are still in flight.
  - Host scatter-adds each core's outT back: out[idx_e] += outT[:, :n_e].T.

Capacity: NCAP tokens/core/round.  If any expert has more assigned tokens
(possible for other routing distributions), the same compiled program runs
additional rounds on the remainder.
"""

import os
import sys

import numpy as np

for _p in ("/opt/trn_rl_repo", "/root/.axon_site/_ro/trn_rl_repo"):
    if os.path.isdir(_p) and _p not in sys.path:
        sys.path.append(_p)

import ml_dtypes

import concourse.bacc as bacc
import concourse.mybir as mybir
import concourse.tile as tile
from concourse.bass_utils import run_bass_kernel_spmd

# Problem shapes (hardcoded per contract).
T, H, F, E, K = 4096, 1024, 2048, 8, 2
NCORES = 8
CKS = [512, 472]     # token chunks (PSUM bank limit: 512 fp32)
NCAP = sum(CKS)      # token capacity per core per round (= seed-wise max)
COFF = [0, 512]
KH = H // 128        # 8  k-tiles over H
KF = F // 128        # 16 k-tiles over F
XLEN = KH * NCAP     # xt dram free length (chunk-major, k-major inside)
XOFF = [0, KH * CKS[0]]  # chunk base offsets in xt dram/sbuf
FP32 = mybir.dt.float32
BF16 = mybir.dt.bfloat16
MUL = mybir.AluOpType.mult
BFNP = ml_dtypes.bfloat16

_PROGRAM = None

# Extra kwargs for run_bass_kernel_spmd — test harness pokes this to enable
# tracing; the grader path leaves it empty.
RUN_KWARGS: dict = {}
LAST_RESULTS = []


def build_program():
    """Build + compile the per-core SPMD FFN program (shared by all cores)."""
    nc = bacc.Bacc(
        "TRN2", target_bir_lowering=False, debug=False, num_devices=NCORES
    )
    # All layouts are exactly the SBUF layouts; host pre-arranges them.
    #   xt:  [p, chunk-major: k-major: token]        (x.T gathered, bf16)
    #   wgu: [p, f-block, {gate,up}, k, m]           (bf16)
    #   wd:  [p, j-block, kf, m]                     (bf16)
    xt_d = nc.dram_tensor("xt", [128, XLEN], BF16, kind="ExternalInput")
    wgu_d = nc.dram_tensor("wgu", [128, KF * 2 * KH * 128], BF16, kind="ExternalInput")
    wd_d = nc.dram_tensor("wd", [128, KH * KF * 128], BF16, kind="ExternalInput")
    yt_d = nc.dram_tensor("yt", [128, KH * NCAP], BF16, kind="ExternalOutput")

    with tile.TileContext(nc) as tc:
        from contextlib import ExitStack

        with ExitStack() as ctx:
            xt_pool = ctx.enter_context(tc.tile_pool(name="xt", bufs=1))
            ht_pool = ctx.enter_context(tc.tile_pool(name="ht", bufs=1))
            wgu_pool = ctx.enter_context(tc.tile_pool(name="wgu", bufs=1))
            wd_pool = ctx.enter_context(tc.tile_pool(name="wd", bufs=1))
            sl_pool = ctx.enter_context(tc.tile_pool(name="sl", bufs=3))
            ob_pool = ctx.enter_context(tc.tile_pool(name="ob", bufs=2))
            wm_pool = ctx.enter_context(tc.tile_pool(name="wm", bufs=1))
            pg_pool = ctx.enter_context(tc.tile_pool(name="pg", bufs=1, space="PSUM"))
            pu_pool = ctx.enter_context(tc.tile_pool(name="pu", bufs=1, space="PSUM"))
            py_pool = ctx.enter_context(tc.tile_pool(name="py", bufs=2, space="PSUM"))

            # PE warm-up: dummy matmuls on a zeroed tile keep the PE busy
            # (and its HAM clock-gate warming) while real DMAs are in flight.
            # They write the same PSUM slot the first real group reuses.
            wm_t = wm_pool.tile([128, 512], BF16, tag="wm")
            nc.gpsimd.memset(wm_t[:], 0)
            # 9 x 512-col dummies ≈ 3.8µs of continuous PE busy: enough to
            # flip the HAM clock-gate (3.4µs window) before real data lands.
            warm_ps = pg_pool.tile([128, 512], FP32, tag="c0")
            for _ in range(9):
                nc.tensor.matmul(
                    warm_ps[:], wm_t[:, :128], wm_t[:], start=True, stop=True
                )

            # All loads ride ONE HWDGE queue (sync) in strict priority order:
            # the SDMA engines round-robin across non-empty queues, so a
            # second live queue would let non-critical bytes starve the
            # first matmul group's data.
            xt_t = xt_pool.tile([128, XLEN], BF16, tag="xt")
            xsplit = [
                (0, 4 * CKS[0]),                    # ci0 k0..3
                (4 * CKS[0], 8 * CKS[0]),           # ci0 k4..7
                (XOFF[1], XOFF[1] + 4 * CKS[1]),    # ci1 k0..3
                (XOFF[1] + 4 * CKS[1], XLEN),       # ci1 k4..7
            ]

            def load_xt(i):
                lo, hi = xsplit[i]
                nc.sync.dma_start(xt_t[:, lo:hi], xt_d.ap()[:, lo:hi])

            def xt_ap(ci, k):
                ck = CKS[ci]
                base = XOFF[ci] + k * ck
                return xt_t[:, base : base + ck]

            # gate/up weights: everything fits in SBUF, so load it ALL
            # up-front in strict priority order — f0 in quarters interleaved
            # with the xt pieces so the first matmuls start early, then the
            # rest in 2-f-block DMAs (fewer DMAs → fewer semaphores → a
            # shorter exit reset cascade).
            FBLK = 2 * KH * 128  # elems per f-block (gate+up)
            wgu_t = wgu_pool.tile([128, KF * FBLK], BF16, tag="wgu")

            def load_wgu(lo, hi):
                nc.sync.dma_start(wgu_t[:, lo:hi], wgu_d.ap()[:, lo:hi])

            def w_ap(f, gu, k):
                base = f * FBLK + (gu * KH + k) * 128
                return wgu_t[:, base : base + 128]

            load_wgu(0, 512)            # f0 gate k0..3
            load_xt(0)                  # ci0 k0..3
            load_wgu(512, 1024)         # f0 gate k4..7
            load_xt(1)                  # ci0 k4..7
            load_wgu(1024, 2048)        # f0 up
            load_xt(2)                  # ci1 k0..3
            load_xt(3)                  # ci1 k4..7
            load_wgu(FBLK, 2 * FBLK)    # f1 solo (just-in-time after f0)
            for f in range(2, KF, 3):   # f2.. in 3-block chunks
                load_wgu(f * FBLK, min(f + 3, KF) * FBLK)

            # wd: two DMAs of 4 j-blocks each, after the gate/up weights.
            wd_t = wd_pool.tile([128, KH * KF * 128], BF16, tag="wd")
            JBLK = KF * 128
            nc.sync.dma_start(wd_t[:, : 4 * JBLK], wd_d.ap()[:, : 4 * JBLK])
            nc.sync.dma_start(wd_t[:, 4 * JBLK :], wd_d.ap()[:, 4 * JBLK :])

            def wd_ap(j, kf):
                base = j * JBLK + kf * 128
                return wd_t[:, base : base + 128]

            # Phase 1: hT[f] = silu(Wg[:,f].T @ xT) * (Wu[:,f].T @ xT)
            # Both token chunks ride each LDWEIGHTS (k-inner, chunk-innermost).
            ht_t = ht_pool.tile([128, KF * NCAP], BF16, tag="ht")

            def ht_ap(kf, ci):
                base = kf * NCAP + COFF[ci]
                return ht_t[:, base : base + CKS[ci]]

            # Chunk-BLOCKED group order: 8 consecutive matmuls per PSUM bank
            # with a weight load per matmul.  (Alternating banks per matmul
            # costs ~26-62ns each on HW; blocked groups run at the ideal
            # N/2.4GHz streaming rate with the loads hidden.)
            for f in range(KF):
                ps = {}
                for ci in range(2):
                    for gu, pool in ((0, pg_pool), (1, pu_pool)):
                        ps[gu, ci] = pool.tile(
                            [128, CKS[ci]], FP32, tag=f"c{ci}",
                            name=f"ps{gu}_{ci}",
                        )
                        for k in range(KH):
                            nc.tensor.matmul(
                                ps[gu, ci][:],
                                w_ap(f, gu, k),
                                xt_ap(ci, k),
                                start=(k == 0),
                                stop=(k == KH - 1),
                            )
                    ck = CKS[ci]
                    sl = sl_pool.tile([128, ck], FP32, tag=f"sl{ci}")
                    nc.scalar.activation(
                        sl[:], ps[0, ci][:], mybir.ActivationFunctionType.Silu
                    )
                    nc.vector.tensor_tensor(
                        ht_ap(f, ci), sl[:], ps[1, ci][:], MUL
                    )

            # Phase 2: yT[j] = Wd[:,j].T @ hT  (coeff applied host-side).
            # Chunk-blocked; each chunk's copy+store drains while the next
            # chunk's matmuls stream.  Final chunk splits fine so the exit
            # tail after the last matmul is short.
            for j in range(KH):
                ob = ob_pool.tile([128, NCAP], BF16, tag="ob")
                last = j == KH - 1
                for ci in range(2):
                    py = py_pool.tile(
                        [128, CKS[ci]], FP32, tag=f"c{ci}", name=f"py{ci}"
                    )
                    for kf in range(KF):
                        nc.tensor.matmul(
                            py[:],
                            wd_ap(j, kf),
                            ht_ap(kf, ci),
                            start=(kf == 0),
                            stop=(kf == KF - 1),
                        )
                    cs, ck = COFF[ci], CKS[ci]
                    if not last:
                        nc.vector.tensor_copy(ob[:, cs : cs + ck], py[:])
                        continue
                    # last j: store each chunk as soon as it's copied, the
                    # final chunk in two pieces so the exit DMA is tiny
                    nsplit = 1 if ci == 0 else 2
                    hw = ck // nsplit
                    for s in range(nsplit):
                        lo = cs + s * hw
                        hi = cs + ck if s == nsplit - 1 else lo + hw
                        nc.vector.tensor_copy(
                            ob[:, lo:hi], py[:, lo - cs : hi - cs]
                        )
                        # last piece rides the (idle) sync queue so the two
                        # exit stores issue in parallel
                        eng = nc.sync if s == nsplit - 1 else nc.scalar
                        eng.dma_start(
                            yt_d.ap()[:, j * NCAP + lo : j * NCAP + hi],
                            ob[:, lo:hi],
                        )
                if not last:
                    # one store per j covering both chunks (fewer DMAs)
                    nc.scalar.dma_start(
                        yt_d.ap()[:, j * NCAP : (j + 1) * NCAP], ob[:]
                    )

    nc.compile()
    return nc


def _get_program():
    global _PROGRAM
    if _PROGRAM is None:
        _PROGRAM = build_program()
    return _PROGRAM


def _prep_weights(gate_weights, up_weights, down_weights):
    """Per-expert bf16 weight blobs in the exact SBUF layouts."""
    wgu_l, wd_l = [], []
    for e in range(NCORES):
        wg4 = gate_weights[e].reshape(KH, 128, KF, 128)
        wu4 = up_weights[e].reshape(KH, 128, KF, 128)
        wgu5 = np.empty((128, KF, 2, KH, 128), dtype=BFNP)
        wgu5[:, :, 0] = wg4.transpose(1, 2, 0, 3)
        wgu5[:, :, 1] = wu4.transpose(1, 2, 0, 3)
        wgu_l.append(np.ascontiguousarray(wgu5.reshape(128, -1)))
        wd4 = down_weights[e].reshape(KF, 128, KH, 128)
        wd_l.append(
            np.ascontiguousarray(
                wd4.transpose(1, 2, 0, 3).astype(BFNP).reshape(128, -1)
            )
        )
    return wgu_l, wd_l


def kernel(x, expert_ids, expert_weights, gate_weights, up_weights, down_weights):
    x = np.asarray(x, dtype=np.float32)
    expert_ids = np.asarray(expert_ids)
    expert_weights = np.asarray(expert_weights, dtype=np.float32)
    gate_weights = np.asarray(gate_weights, dtype=np.float32)
    up_weights = np.asarray(up_weights, dtype=np.float32)
    down_weights = np.asarray(down_weights, dtype=np.float32)

    t_dim, h_dim = x.shape
    n_exp = gate_weights.shape[0]
    assert h_dim == H and gate_weights.shape[1:] == (H, F), (
        "program compiled for H=1024, F=2048"
    )
    assert n_exp == NCORES, "expert-parallel mapping assumes E == 8 cores"

    # Routing table: per-token combined coefficient per expert.
    coeff = np.zeros((t_dim, n_exp), np.float32)
    rows = np.arange(t_dim)
    for k in range(expert_ids.shape[1]):
        np.add.at(coeff, (rows, expert_ids[:, k]), expert_weights[:, k])

    idx_per_e = [np.nonzero(coeff[:, e])[0] for e in range(n_exp)]
    rounds = max(1, max((len(i) + NCAP - 1) // NCAP for i in idx_per_e))

    wgu_l, wd_l = _prep_weights(gate_weights, up_weights, down_weights)
    xT = np.ascontiguousarray(x.T)  # [H, T]
    nc = _get_program()

    out = np.zeros((t_dim, h_dim), np.float32)
    LAST_RESULTS.clear()
    for r in range(rounds):
        in_maps = []
        idx_r_per_e = []
        for e in range(n_exp):
            idx_r = idx_per_e[e][r * NCAP : (r + 1) * NCAP]
            idx_r_per_e.append(idx_r)
            xe = np.zeros((h_dim, NCAP), np.float32)
            if len(idx_r):
                xe[:, : len(idx_r)] = xT[:, idx_r]
            x3 = xe.reshape(KH, 128, NCAP)
            xt_arr = np.empty((128, XLEN), dtype=BFNP)
            xt_arr[:, : XOFF[1]] = (
                x3[:, :, : CKS[0]].transpose(1, 0, 2).reshape(128, -1)
            )
            xt_arr[:, XOFF[1] :] = (
                x3[:, :, CKS[0] :].transpose(1, 0, 2).reshape(128, -1)
            )
            in_maps.append(
                {
                    "xt": xt_arr,
                    "wgu": wgu_l[e],
                    "wd": wd_l[e],
                }
            )
        res = run_bass_kernel_spmd(
            nc, in_maps, core_ids=list(range(NCORES)), **RUN_KWARGS
        )
        LAST_RESULTS.append(res)
        for e in range(n_exp):
            idx_r = idx_r_per_e[e]
            if len(idx_r):
                yt = res.results[e]["yt"]  # [128, KH*NCAP] bf16, unscaled
                y3 = (
                    yt.reshape(128, KH, NCAP)
                    .astype(np.float32)
                    .transpose(1, 0, 2)
                    .reshape(h_dim, NCAP)
                )
                out[idx_r, :] += coeff[idx_r, e][:, None] * y3[:, : len(idx_r)].T
    return out
